# revision 8
# baseline (speedup 1.0000x reference)
"""Trainium2 Bass kernel for CommittorNetBP (pairwise min-image env sum + tiny MLP).

Algorithm (mathematically equivalent reformulation of the reference, validated
to ~1e-4 max rel err in fp32):

 1. Per-component wrapped squared displacement is periodic in dx with period
    L=10, so  wrap(dx)^2 ~= B0 + sum_n Bn cos(2*pi*n*dx/L)  (Chebyshev/Fourier
    fit, N=16 harmonics, accurate on |dx| <= L/4 which covers the cutoff).
    Hence d2[i,j] = sum_k wrap2(dx_k) is an inner product of trig embeddings:
    one TensorEngine matmul per tile block.
 2. The envelope f(t) = exp(-t)*0.5*(1+cos(pi*sqrt(t)/RC)) (t=d2, cut at
    t>=RC^2) is approximated by a 3-term exponential sum  sum_r w_r e^{-a_r t}
    -> only Exp activations (single ACT table set, no sqrt/cos chain).
 3. Row sums  sum_j w_r e_r[i,j]  are ones-matmuls on the TensorEngine with
    w_r baked into the stationary column; the j==i diagonal (f(0)=1, enforced
    exactly by sum_r w_r = 1 and B(0)=0) is folded into the MLP bias:
    b1' = b1 - W1 @ 1.
 4. MLP: h = relu(inputt @ W1.T + b1'), out = sigmoid(h @ W2.T) computed as
    0.5 + 0.5*tanh(z/2) (tanh shares the exp ACT table set).

Sharding: pure data parallel, batch 128 -> 8 cores x 16.
"""

import numpy as np

# ---------------------------------------------------------------- constants
L = 10.0
RC = 2.5
PI = float(np.pi)
NP = 512
BTOT = 128
NCORES = 8
BLOC = BTOT // NCORES  # 16
NH = 16                # harmonics
K = 6 * NH + 1         # 97 embedding rows
R = 3                  # exponential-sum terms
NUM_NODES = 256

# fitted coefficients (see fit.py): wrap2(theta) ~= sum_n B[n] cos(n theta)
B_HARM = [
    8.336507198660753, -10.134305777836879, 2.5283072633082164,
    -1.1207547738471013, 0.6351791173907125, -0.41237594667899846,
    0.28478810229590223, -0.20163605059415754, 0.15059719920404221,
    -0.12490354747428888, 0.11118898587488348, -0.09477489833163562,
    0.06985971056432684, -0.041620415059490684, 0.018837434788739185,
    -0.005869820105041354, 0.0009762178400180537,
]
ALPHAS = [1.0349286136376832, 1.1463391199463473, 0.9940976027139273]
WS = [-10.377784817895446, 5.413830282268338, 5.963954535627107]

f32 = np.float32


def _host_constants():
    """Mturns [4,K], Bcol [K,1], wcol [128,R], eye16 [16,16]."""
    mt = np.zeros((4, K), f32)
    bcol = np.zeros((K, 1), f32)
    mt[3, 0] = 0.25            # const row: sin(2*pi*0.25) = 1
    bcol[0, 0] = 3.0 * B_HARM[0]
    col = 1
    for k in range(3):
        for n in range(1, NH + 1):
            mt[k, col] = n / L      # cos component (phase 0.25 turns)
            mt[3, col] = 0.25
            bcol[col, 0] = B_HARM[n]
            col += 1
            mt[k, col] = n / L      # sin component (phase 0)
            mt[3, col] = 0.0
            bcol[col, 0] = B_HARM[n]
            col += 1
    wcol = np.zeros((128, R), f32)
    for r in range(R):
        wcol[:, r] = WS[r]
    eye16 = np.eye(16, dtype=f32)
    return mt, bcol, wcol, eye16


_CACHE = {}


def _build_program():
    import concourse.bacc as bacc
    import concourse.mybir as mybir
    import concourse.tile as tile

    nc = bacc.Bacc("TRN2", target_bir_lowering=False, debug=False,
                   num_devices=NCORES)
    dt = mybir.dt
    AF = mybir.ActivationFunctionType
    ALU = mybir.AluOpType

    xa_d = nc.declare_dram_parameter("xa", (4, BLOC * NP), dt.float32, isOutput=False)
    mt_d = nc.declare_dram_parameter("mt", (4, K), dt.float32, isOutput=False)
    bcol_d = nc.declare_dram_parameter("bcol", (K, 1), dt.float32, isOutput=False)
    wcol_d = nc.declare_dram_parameter("wcol", (128, R), dt.float32, isOutput=False)
    w1t_d = nc.declare_dram_parameter("w1t", (NP, NUM_NODES), dt.float32, isOutput=False)
    b1p_d = nc.declare_dram_parameter("b1p", (1, NUM_NODES), dt.float32, isOutput=False)
    w2r_d = nc.declare_dram_parameter("w2r", (BLOC, NUM_NODES), dt.float32, isOutput=False)
    eye_d = nc.declare_dram_parameter("eye16", (16, 16), dt.float32, isOutput=False)
    y_d = nc.declare_dram_parameter("y", (BLOC, 1), dt.float32, isOutput=True)

    with tile.TileContext(nc) as tc:
        with tc.tile_pool(name="const", bufs=1) as cpool:
            xa_s = cpool.tile([4, BLOC * NP], dt.float32)
            nc.gpsimd.dma_start(xa_s[:], xa_d[:])
            mt_s = cpool.tile([4, K], dt.float32)
            nc.gpsimd.dma_start(mt_s[:], mt_d[:])
            bcol_s = cpool.tile([K, 1], dt.float32)
            nc.gpsimd.dma_start(bcol_s[:], bcol_d[:])
            wcol_s = cpool.tile([128, R], dt.float32)
            nc.gpsimd.dma_start(wcol_s[:], wcol_d[:])
            w1t_s = cpool.tile([128, 4 * NUM_NODES], dt.float32)
            for c in range(4):
                nc.gpsimd.dma_start(
                    w1t_s[:, c * NUM_NODES:(c + 1) * NUM_NODES],
                    w1t_d[c * 128:(c + 1) * 128, :])
            b1p_s = cpool.tile([1, NUM_NODES], dt.float32)
            nc.gpsimd.dma_start(b1p_s[:], b1p_d[:])
            w2r_s = cpool.tile([BLOC, NUM_NODES], dt.float32)
            nc.gpsimd.dma_start(w2r_s[:], w2r_d[:])
            eye_s = cpool.tile([16, 16], dt.float32)
            nc.gpsimd.dma_start(eye_s[:], eye_d[:])
            ones1_s = cpool.tile([1, BLOC], dt.float32)
            nc.gpsimd.memset(ones1_s[:], 1.0)

            # ---------------- phase 1: trig embeddings per batch ----------------
            with (
                tc.tile_pool(name="upsum", bufs=2, space="PSUM") as upsum,
                tc.tile_pool(name="ri", bufs=2) as ripool,
                tc.tile_pool(name="vv", bufs=2) as vpool,
                tc.tile_pool(name="E", bufs=BLOC) as epool,
                tc.tile_pool(name="Ew", bufs=BLOC) as ewpool,
            ):
                E_l, Ew_l = [], []
                for b in range(BLOC):
                    u = upsum.tile([K, NP], dt.float32)
                    nc.tensor.matmul(u[:], mt_s[:], xa_s[:, b * NP:(b + 1) * NP],
                                     start=True, stop=True)
                    ri = ripool.tile([K, NP], dt.int32)
                    nc.vector.tensor_copy(ri[:], u[:])          # round to nearest
                    v = vpool.tile([K, NP], dt.float32)
                    nc.vector.tensor_tensor(v[:], u[:], ri[:], ALU.subtract)
                    E = epool.tile([K, NP], dt.float32, tag="E")
                    nc.scalar.activation(E[:], v[:], AF.Sin, scale=2.0 * PI)
                    Ew = ewpool.tile([K, NP], dt.float32, tag="Ew")
                    nc.vector.tensor_scalar(Ew[:], E[:], bcol_s[:, 0:1], None, ALU.mult)
                    E_l.append(E)
                    Ew_l.append(Ew)

                # keep all Sin ops ahead of all Exp ops in the ACT stream
                # (sin and exp live in different ACT table sets).
                tc.no_sync_barrier()

                # ---------------- phase 2: pair blocks ----------------
                scopy = cpool.tile([BLOC, NP], dt.float32)
                with (
                    tc.tile_pool(name="spsum", bufs=2, space="PSUM") as spsum,
                    tc.tile_pool(name="ssb", bufs=2) as ssbpool,
                    tc.tile_pool(name="tpsum", bufs=2, space="PSUM") as tpsum,
                    tc.tile_pool(name="er", bufs=2 * R) as erpool,
                ):
                    for b in range(BLOC):
                        srow = spsum.tile([1, NP], dt.float32, tag="srow")
                        for g in range(2):
                            t = tpsum.tile([128, 2 * NP], dt.float32, tag="t")
                            for jj in range(2):
                                jc = 2 * g + jj
                                nc.tensor.matmul(
                                    t[:, jj * NP:(jj + 1) * NP],
                                    Ew_l[b][:, jc * 128:(jc + 1) * 128],
                                    E_l[b][:],
                                    start=True, stop=True)
                            for r in range(R):
                                er = erpool.tile([128, 2 * NP], dt.float32, tag="er")
                                nc.scalar.activation(er[:], t[:], AF.Exp,
                                                     scale=-ALPHAS[r])
                                for jj in range(2):
                                    first = (g == 0 and r == 0 and jj == 0)
                                    last = (g == 1 and r == R - 1 and jj == 1)
                                    nc.tensor.matmul(
                                        srow[:],
                                        wcol_s[:, r:r + 1],
                                        er[:, jj * NP:(jj + 1) * NP],
                                        start=first, stop=last,
                                        skip_group_check=True)
                        ssb = ssbpool.tile([1, NP], dt.float32, tag="ssb")
                        nc.vector.tensor_copy(ssb[:], srow[:])
                        nc.gpsimd.dma_start(scopy[b:b + 1, :], ssb[:])

                with (
                    tc.tile_pool(name="trpsum", bufs=2, space="PSUM") as trpsum,
                    tc.tile_pool(name="hpsum", bufs=1, space="PSUM") as hpsum,
                    tc.tile_pool(name="tail", bufs=1) as tail,
                ):
                    it_l = []
                    for c in range(4):
                        tp = trpsum.tile([128, BLOC], dt.float32, tag="tp")
                        nc.tensor.transpose(tp[:], scopy[:, c * 128:(c + 1) * 128],
                                            eye_s[:])
                        it = tail.tile([128, BLOC], dt.float32, tag=f"it{c}")
                        nc.vector.tensor_copy(it[:], tp[:])
                        it_l.append(it)
                    h = hpsum.tile([BLOC, NUM_NODES], dt.float32)
                    for c in range(4):
                        nc.tensor.matmul(h[:], it_l[c][:],
                                         w1t_s[:, c * NUM_NODES:(c + 1) * NUM_NODES],
                                         start=(c == 0), stop=False)
                    nc.tensor.matmul(h[:], ones1_s[:], b1p_s[:],
                                     start=False, stop=True)
                    hr = tail.tile([BLOC, NUM_NODES], dt.float32)
                    nc.scalar.activation(hr[:], h[:], AF.Relu)
                    hw = tail.tile([BLOC, NUM_NODES], dt.float32)
                    nc.vector.tensor_tensor(hw[:], hr[:], w2r_s[:], ALU.mult)
                    z = tail.tile([BLOC, 1], dt.float32)
                    nc.vector.reduce_sum(z[:], hw[:], axis=mybir.AxisListType.X)
                    th = tail.tile([BLOC, 1], dt.float32)
                    nc.scalar.activation(th[:], z[:], AF.Tanh, scale=0.5)
                    ys = tail.tile([BLOC, 1], dt.float32)
                    nc.vector.tensor_scalar(ys[:], th[:], 0.5, 0.5,
                                            ALU.mult, ALU.add)
                    nc.gpsimd.dma_start(y_d[:], ys[:])

    nc.finalize()
    return nc


def _get_program():
    if "nc" not in _CACHE:
        _CACHE["nc"] = _build_program()
    return _CACHE["nc"]


def _make_in_maps(x, W1, b1, W2):
    mt, bcol, wcol, eye16 = _host_constants()
    W1 = np.asarray(W1, f32)
    w1t = np.ascontiguousarray(W1.T)
    b1p = (np.asarray(b1, f32) - W1.sum(axis=1)).reshape(1, NUM_NODES).astype(f32)
    w2r = np.broadcast_to(np.asarray(W2, f32).reshape(1, NUM_NODES),
                          (BLOC, NUM_NODES)).copy()
    x = np.asarray(x, f32)
    in_maps = []
    for c in range(NCORES):
        xs = x[c * BLOC:(c + 1) * BLOC]                     # [16,512,3]
        xT = np.transpose(xs, (2, 0, 1)).reshape(3, BLOC * NP)  # [3,16*512]
        xa = np.concatenate([xT, np.ones((1, BLOC * NP), f32)], axis=0)
        in_maps.append({
            "xa": np.ascontiguousarray(xa),
            "mt": mt, "bcol": bcol, "wcol": wcol,
            "w1t": w1t, "b1p": b1p, "w2r": w2r, "eye16": eye16,
        })
    return in_maps


def kernel(x, W1, b1, W2, _trace=False, _trace_kwargs=None):
    from concourse.bass_utils import run_bass_kernel_spmd

    nc = _get_program()
    in_maps = _make_in_maps(x, W1, b1, W2)
    res = run_bass_kernel_spmd(nc, in_maps, list(range(NCORES)),
                               trace=_trace, **(_trace_kwargs or {}))
    out = np.concatenate([res.results[c]["y"] for c in range(NCORES)], axis=0)
    if _trace:
        _CACHE["last_result"] = res
    return out.astype(f32)


# revision 12
# speedup vs baseline: 1.0752x; 1.0752x over previous
"""Trainium2 Bass kernel for CommittorNetBP (pairwise min-image env sum + tiny MLP).

Algorithm (mathematically equivalent reformulation of the reference, validated
to ~1e-4 max rel err in fp32):

 1. Per-component wrapped squared displacement is periodic in dx with period
    L=10, so  wrap(dx)^2 ~= B0 + sum_n Bn cos(2*pi*n*dx/L)  (Chebyshev/Fourier
    fit, N=16 harmonics, accurate on |dx| <= L/4 which covers the cutoff).
    Hence d2[i,j] = sum_k wrap2(dx_k) is an inner product of trig embeddings:
    one TensorEngine matmul per tile block.
 2. The envelope f(t) = exp(-t)*0.5*(1+cos(pi*sqrt(t)/RC)) (t=d2, cut at
    t>=RC^2) is approximated by a 3-term exponential sum  sum_r w_r e^{-a_r t}
    -> only Exp activations (single ACT table set, no sqrt/cos chain).
 3. Row sums  sum_j w_r e_r[i,j]  are ones-matmuls on the TensorEngine with
    w_r baked into the stationary column; the j==i diagonal (f(0)=1, enforced
    exactly by sum_r w_r = 1 and B(0)=0) is folded into the MLP bias:
    b1' = b1 - W1 @ 1.
 4. MLP: h = relu(inputt @ W1.T + b1'), out = sigmoid(h @ W2.T) computed as
    0.5 + 0.5*tanh(z/2) (tanh shares the exp ACT table set).

Sharding: pure data parallel, batch 128 -> 8 cores x 16.
"""

import numpy as np

# ---------------------------------------------------------------- constants
L = 10.0
RC = 2.5
PI = float(np.pi)
NP = 512
BTOT = 128
NCORES = 8
BLOC = BTOT // NCORES  # 16
NH = 16                # harmonics
K = 6 * NH + 1         # 97 embedding rows
R = 3                  # exponential-sum terms
NUM_NODES = 256

# fitted coefficients (see fit.py): wrap2(theta) ~= sum_n B[n] cos(n theta)
B_HARM = [
    8.336507198660753, -10.134305777836879, 2.5283072633082164,
    -1.1207547738471013, 0.6351791173907125, -0.41237594667899846,
    0.28478810229590223, -0.20163605059415754, 0.15059719920404221,
    -0.12490354747428888, 0.11118898587488348, -0.09477489833163562,
    0.06985971056432684, -0.041620415059490684, 0.018837434788739185,
    -0.005869820105041354, 0.0009762178400180537,
]
ALPHAS = [1.0349286136376832, 1.1463391199463473, 0.9940976027139273]
WS = [-10.377784817895446, 5.413830282268338, 5.963954535627107]

f32 = np.float32


def _host_constants():
    """Mturns [4,K], Bcol [K,1]."""
    mt = np.zeros((4, K), f32)
    bcol = np.zeros((K, 1), f32)
    mt[3, 0] = 0.25            # const row: sin(2*pi*0.25) = 1
    bcol[0, 0] = 3.0 * B_HARM[0]
    col = 1
    for k in range(3):
        for n in range(1, NH + 1):
            mt[k, col] = n / L      # cos component (phase 0.25 turns)
            mt[3, col] = 0.25
            bcol[col, 0] = B_HARM[n]
            col += 1
            mt[k, col] = n / L      # sin component (phase 0)
            mt[3, col] = 0.0
            bcol[col, 0] = B_HARM[n]
            col += 1
    return mt, bcol


_CACHE = {}


def _build_program():
    import concourse.bacc as bacc
    import concourse.mybir as mybir
    import concourse.tile as tile

    nc = bacc.Bacc("TRN2", target_bir_lowering=False, debug=False,
                   num_devices=NCORES)
    dt = mybir.dt
    AF = mybir.ActivationFunctionType
    ALU = mybir.AluOpType

    xa_d = nc.declare_dram_parameter("xa", (4, BLOC * NP), dt.float32, isOutput=False)
    mt_d = nc.declare_dram_parameter("mt", (4, K), dt.float32, isOutput=False)
    bcol_d = nc.declare_dram_parameter("bcol", (K, 1), dt.float32, isOutput=False)
    w1t_d = nc.declare_dram_parameter("w1t", (NP, NUM_NODES), dt.float32, isOutput=False)
    b1p_d = nc.declare_dram_parameter("b1p", (1, NUM_NODES), dt.float32, isOutput=False)
    w2r_d = nc.declare_dram_parameter("w2r", (BLOC, NUM_NODES), dt.float32, isOutput=False)
    y_d = nc.declare_dram_parameter("y", (BLOC, 1), dt.float32, isOutput=True)

    with tile.TileContext(nc) as tc:
        with tc.tile_pool(name="const", bufs=1) as cpool:
            xa_s = cpool.tile([4, BLOC * NP], dt.float32)
            nc.gpsimd.dma_start(xa_s[:], xa_d[:])
            mt_s = cpool.tile([4, K], dt.float32)
            nc.gpsimd.dma_start(mt_s[:], mt_d[:])
            bcol_s = cpool.tile([K, 1], dt.float32)
            nc.gpsimd.dma_start(bcol_s[:], bcol_d[:])
            w1t_s = cpool.tile([128, 4 * NUM_NODES], dt.float32)
            for c in range(4):
                nc.gpsimd.dma_start(
                    w1t_s[:, c * NUM_NODES:(c + 1) * NUM_NODES],
                    w1t_d[c * 128:(c + 1) * 128, :])
            b1p_s = cpool.tile([1, NUM_NODES], dt.float32)
            nc.gpsimd.dma_start(b1p_s[:], b1p_d[:])
            w2r_s = cpool.tile([BLOC, NUM_NODES], dt.float32)
            nc.gpsimd.dma_start(w2r_s[:], w2r_d[:])
            ones1_s = cpool.tile([1, BLOC], dt.float32)
            nc.gpsimd.memset(ones1_s[:], 1.0)

            # ---------------- phase 1: trig embeddings per batch ----------------
            with (
                tc.tile_pool(name="upsum", bufs=2, space="PSUM") as upsum,
                tc.tile_pool(name="ri", bufs=2) as ripool,
                tc.tile_pool(name="vv", bufs=2) as vpool,
                tc.tile_pool(name="E", bufs=BLOC) as epool,
                tc.tile_pool(name="Ew", bufs=BLOC) as ewpool,
            ):
                E_l, Ew_l = [], []
                for b in range(BLOC):
                    u = upsum.tile([K, NP], dt.float32)
                    nc.tensor.matmul(u[:], mt_s[:], xa_s[:, b * NP:(b + 1) * NP],
                                     start=True, stop=True)
                    ri = ripool.tile([K, NP], dt.int32)
                    nc.vector.tensor_copy(ri[:], u[:])          # round to nearest
                    v = vpool.tile([K, NP], dt.float32)
                    nc.vector.tensor_tensor(v[:], u[:], ri[:], ALU.subtract)
                    E = epool.tile([K, NP], dt.float32, tag="E")
                    nc.scalar.activation(E[:], v[:], AF.Sin, scale=2.0 * PI)
                    Ew = ewpool.tile([K, NP], dt.float32, tag="Ew")
                    nc.vector.tensor_scalar(Ew[:], E[:], bcol_s[:, 0:1], None, ALU.mult)
                    E_l.append(E)
                    Ew_l.append(Ew)

                # keep all Sin ops ahead of all Exp ops in the ACT stream
                # (sin and exp live in different ACT table sets).
                tc.no_sync_barrier()

                # ---------------- phase 2: pair blocks ----------------
                with (
                    tc.tile_pool(name="acc", bufs=4 * R) as accpool,
                    tc.tile_pool(name="tpsum", bufs=4, space="PSUM") as tpsum,
                    tc.tile_pool(name="er", bufs=3) as erpool,
                ):
                    # acc[jc][r][p, b] = sum_j exp(-alpha_r * d2a[jc*128+p, j])
                    acc = [[accpool.tile([128, BLOC], dt.float32,
                                         name=f"acc{jc}_{r}", tag=f"a{jc}_{r}")
                            for r in range(R)] for jc in range(4)]
                    for b in range(BLOC):
                        for jc in range(4):
                            t = tpsum.tile([128, NP], dt.float32, tag="t")
                            nc.tensor.matmul(
                                t[:],
                                Ew_l[b][:, jc * 128:(jc + 1) * 128],
                                E_l[b][:],
                                start=True, stop=True)
                            for r in range(R):
                                er = erpool.tile([128, NP], dt.float32, tag="er")
                                nc.scalar.activation(
                                    er[:], t[:], AF.Exp, scale=-ALPHAS[r],
                                    accum_out=acc[jc][r][:, b:b + 1])

                    # inputt chunks (transposed layout): it = sum_r w_r acc_r
                    it_l = []
                    for jc in range(4):
                        zs = []
                        for r in range(R):
                            z = accpool.tile([128, BLOC], dt.float32, tag=f"z{r}")
                            nc.vector.tensor_scalar(z[:], acc[jc][r][:], WS[r],
                                                    None, ALU.mult)
                            zs.append(z)
                        it = cpool.tile([128, BLOC], dt.float32, tag=f"it{jc}")
                        nc.vector.tensor_tensor(it[:], zs[0][:], zs[1][:], ALU.add)
                        for r in range(2, R):
                            nc.vector.tensor_tensor(it[:], it[:], zs[r][:], ALU.add)
                        it_l.append(it)

                with (
                    tc.tile_pool(name="hpsum", bufs=1, space="PSUM") as hpsum,
                    tc.tile_pool(name="tail", bufs=1) as tail,
                ):
                    h = hpsum.tile([BLOC, NUM_NODES], dt.float32)
                    for c in range(4):
                        nc.tensor.matmul(h[:], it_l[c][:],
                                         w1t_s[:, c * NUM_NODES:(c + 1) * NUM_NODES],
                                         start=(c == 0), stop=False)
                    nc.tensor.matmul(h[:], ones1_s[:], b1p_s[:],
                                     start=False, stop=True)
                    hr = tail.tile([BLOC, NUM_NODES], dt.float32)
                    nc.scalar.activation(hr[:], h[:], AF.Relu)
                    hw = tail.tile([BLOC, NUM_NODES], dt.float32)
                    nc.vector.tensor_tensor(hw[:], hr[:], w2r_s[:], ALU.mult)
                    z = tail.tile([BLOC, 1], dt.float32)
                    nc.vector.reduce_sum(z[:], hw[:], axis=mybir.AxisListType.X)
                    th = tail.tile([BLOC, 1], dt.float32)
                    nc.scalar.activation(th[:], z[:], AF.Tanh, scale=0.5)
                    ys = tail.tile([BLOC, 1], dt.float32)
                    nc.vector.tensor_scalar(ys[:], th[:], 0.5, 0.5,
                                            ALU.mult, ALU.add)
                    nc.gpsimd.dma_start(y_d[:], ys[:])

    nc.finalize()
    return nc


def _get_program():
    if "nc" not in _CACHE:
        _CACHE["nc"] = _build_program()
    return _CACHE["nc"]


def _make_in_maps(x, W1, b1, W2):
    mt, bcol = _host_constants()
    W1 = np.asarray(W1, f32)
    w1t = np.ascontiguousarray(W1.T)
    b1p = (np.asarray(b1, f32) - W1.sum(axis=1)).reshape(1, NUM_NODES).astype(f32)
    w2r = np.broadcast_to(np.asarray(W2, f32).reshape(1, NUM_NODES),
                          (BLOC, NUM_NODES)).copy()
    x = np.asarray(x, f32)
    in_maps = []
    for c in range(NCORES):
        xs = x[c * BLOC:(c + 1) * BLOC]                     # [16,512,3]
        xT = np.transpose(xs, (2, 0, 1)).reshape(3, BLOC * NP)  # [3,16*512]
        xa = np.concatenate([xT, np.ones((1, BLOC * NP), f32)], axis=0)
        in_maps.append({
            "xa": np.ascontiguousarray(xa),
            "mt": mt, "bcol": bcol,
            "w1t": w1t, "b1p": b1p, "w2r": w2r,
        })
    return in_maps


def kernel(x, W1, b1, W2, _trace=False, _trace_kwargs=None):
    from concourse.bass_utils import run_bass_kernel_spmd

    nc = _get_program()
    in_maps = _make_in_maps(x, W1, b1, W2)
    res = run_bass_kernel_spmd(nc, in_maps, list(range(NCORES)),
                               trace=_trace, **(_trace_kwargs or {}))
    out = np.concatenate([res.results[c]["y"] for c in range(NCORES)], axis=0)
    if _trace:
        _CACHE["last_result"] = res
    return out.astype(f32)
